# revision 63
# baseline (speedup 1.0000x reference)
"""Multi-head causal attention on 8 Trainium2 NeuronCores.

nn_MultiHeadAttention_37933151158277: x[2,2048,2048] f32, causal mask,
W_qkv[6144,2048], W_o[2048,2048]. Tensor-parallel over heads (2 per
core), per the sharding hint: qkv_proj output and W_o input are split
along the head dimension; x is replicated. Each core:

  phase 1 - QKV projection. Host supplies x^T [D, T] and per-core
      weight slices pre-transposed (all bf16), so Q^T/K^T land as
      [d_k=128, tok] and V as [tok, d_k] with zero on-device
      transposes. Weights are SBUF-resident; x^T streams through in
      [128, 512] tiles feeding 8 accumulating PSUM banks. DMA issues
      cost ~650ns of serial queue time each, so weights ride the sync
      (SP) HWDGE queue while x^T rides the scalar (ACT) queue, with
      the first-needed tiles split in half across two DMA engines.
  phase 2 - attention per (batch, head). Scores are computed
      transposed: S^T[k, q] = K^T_tile.T @ Q^T (contraction over d), so
      the P @ V matmul consumes exp(S^T) directly with V tiles as the
      stationary operand. No max-subtraction (scores are O(1) by
      construction, exp cannot overflow). exp partial sums accumulate
      on two chains (DVE + GPSIMD) as each e-tile is produced; the
      q-block tail (merge, K=1 ones matmul for the cross-partition
      sum, custom-DVE reciprocal_approx_fast, K=1 broadcast matmul,
      o*(1/d)) is software-pipelined into the NEXT block's matmul
      rounds - including the last PV, whose exp is otherwise only one
      ACT slot old - so the in-order PE queue never waits on the
      ACT/DVE burst at block boundaries. Head-streams are emitted
      entry-interleaved; exp runs on 2-k-tile PSUM groups.
  phase 3 - partial out-projection y_c = attn_out @ W_o[:, cols]^T in
      bf16, drained one tile per attention round from a queue so proj
      matmuls always have ready inputs; y partials stream out in bf16
      on the sync queue only (the ACT queue must stay free for the
      exp cadence), with the closing tiles split across both queues.

Host: y = sum_c y_c (f32 accumulation of the bf16 partials).

All big matmuls run bf16 (same 1 cycle/row PE rate as f32r but half
the LDWEIGHTS/SBUF bandwidth); PSUM accumulation and the softmax
denominator chain stay f32/f32r. The mask is analyzed block-wise at
trace time: fully-masked blocks are skipped, fully-valid blocks skip
the mask multiply, mixed blocks get a (content-deduped) DMA'd
mask-tile multiply.

Measured: ~379 us on hardware per core (8 cores SPMD), rel err 4.3e-3
scale-relative absmax (baseline at f32r: ~472 us, 2.5e-4).
"""
import sys
if '/opt/trn_rl_repo' not in sys.path:
    sys.path.insert(0, '/opt/trn_rl_repo')

import numpy as np

B, S, D = 2, 2048, 2048
H, DK = 16, 128
NCORES = 8
HPC = H // NCORES            # heads per core
T = B * S                    # tokens
QB = 512                     # q-block width (free dim of S^T / PV matmuls)
NKT = S // 128               # k tiles per batch (16)
NQB = S // QB                # q blocks per batch (4)
NCH = T // QB                # token chunks (8)
NDT = D // 128               # d_model tiles (16)
SGRP = 1                     # k-tiles per s-psum group

_cache = {}


def _analyze_mask(m2):
    """m2: [S, S] bool, m2[q, k]. Returns blocks[qb] = list of entries
    (j, q0c, mm0, mm1) ascending j:
      q0c: first q col (within block) to compute, mm0..mm1: mask-mul range
      (None if block fully valid over [q0c, QB)).
    """
    blocks = []
    for qb in range(NQB):
        entries = []
        for j in range(NKT):
            blk = m2[qb * QB:(qb + 1) * QB, j * 128:(j + 1) * 128]
            col_any = blk.any(axis=1)
            if not col_any.any():
                continue
            col_all = blk.all(axis=1)
            q0 = int(np.argmax(col_any))
            # q1: start of the trailing fully-valid run
            rev = col_all[::-1]
            run = int(np.argmin(rev)) if not rev.all() else QB
            q1 = QB - run
            if q1 <= q0:
                entries.append((j, q0, None, None))
            else:
                entries.append((j, q0, q0, q1))
        if entries:
            qmin = min(e[1] for e in entries)
            j, q0, m0, m1 = entries[0]
            if q0 > qmin:
                # first entry must cover every column later entries write
                entries[0] = (j, qmin, qmin, m1 if m1 is not None else q0)
        blocks.append(entries)
    return blocks


def _build(mask_bool):
    from contextlib import ExitStack
    import concourse.bass as bass
    import concourse.tile as tile
    from concourse import bacc, mybir

    f32 = mybir.dt.float32
    f32r = mybir.dt.float32r
    bf16 = mybir.dt.bfloat16
    EXP = mybir.ActivationFunctionType.Exp
    scale = 1.0 / np.sqrt(DK)

    m2 = mask_bool
    blocks = _analyze_mask(m2)

    nc = bacc.Bacc("TRN2", target_bir_lowering=False, debug=False)
    xt_d = nc.dram_tensor("xt", [D, T], bf16, kind="ExternalInput")
    wqk_d = nc.dram_tensor("wqk", [D, 4 * 128], bf16, kind="ExternalInput")
    wv_d = nc.dram_tensor("wv", [D, 2 * 128], bf16, kind="ExternalInput")
    wo_d = nc.dram_tensor("wo", [2 * 128, D], bf16, kind="ExternalInput")
    mt_d = nc.dram_tensor("mt", [S, S], bf16, kind="ExternalInput")
    y_d = nc.dram_tensor("y", [T, D], bf16, kind="ExternalOutput")

    with tile.TileContext(nc) as tc:
        with ExitStack() as stack:
            stack.enter_context(
                nc.allow_low_precision(reason="float32r matmul inputs"))
            qkt_pool = stack.enter_context(tc.tile_pool(name="qkt", bufs=1))
            v_pool = stack.enter_context(tc.tile_pool(name="vsb", bufs=1))
            att_pool = stack.enter_context(tc.tile_pool(name="att", bufs=1))

            # persistent SBUF
            qt_sb = [qkt_pool.tile([128, T], bf16, tag=f"qt{h}", name=f"qt{h}")
                     for h in range(HPC)]
            kt_sb = [qkt_pool.tile([128, T], bf16, tag=f"kt{h}", name=f"kt{h}")
                     for h in range(HPC)]
            v_sb = v_pool.tile([128, (T // 128) * 256], bf16, tag="v")
            at_sb = [att_pool.tile([128, T], bf16, tag=f"at{h}", name=f"at{h}")
                     for h in range(HPC)]

            wo_pool = stack.enter_context(tc.tile_pool(name="wo", bufs=1))
            wo_sb = [wo_pool.tile([128, D], bf16, tag=f"wo{h}", name=f"wo{h}")
                     for h in range(HPC)]

            cst_pool = stack.enter_context(tc.tile_pool(name="cst", bufs=1))
            ones_f = cst_pool.tile([128, 128], f32, tag="ones_f")
            nc.vector.memset(ones_f[:], 1.0)
            ones_col = cst_pool.tile([128, 1], f32r, tag="ones_c")
            nc.scalar.copy(ones_col[:], ones_f[:, 0:1])
            ones_row = cst_pool.tile([1, 128], f32r, tag="ones_r")
            nc.scalar.copy(ones_row[:], ones_f[0:1, :])

            # ---------------- phase 1: QKV projection ----------------
            # DMA issue costs ~650ns/call serially per issue queue; weights
            # interleave on the sync (SP) queue while xt streams on the
            # scalar (ACT) HWDGE queue so the first matmul's inputs land
            # ~7us in instead of behind a 35-DMA weight convoy. wo (needed
            # only in phase 3) issues after the phase-1-critical weights.
            with ExitStack() as p1:
                wqk_pool = p1.enter_context(tc.tile_pool(name="wqk", bufs=1))
                wv_pool = p1.enter_context(tc.tile_pool(name="wv", bufs=1))
                xt_pool = p1.enter_context(tc.tile_pool(name="xt", bufs=10))
                qk_ps_pool = p1.enter_context(
                    tc.tile_pool(name="ps_qk", bufs=4, space="PSUM"))
                v_ps_pool = p1.enter_context(
                    tc.tile_pool(name="ps_v", bufs=4, space="PSUM"))

                wqk_sb = []
                wv_sb = []
                for kd in range(NDT):
                    wq = wqk_pool.tile([128, 512], bf16, tag=f"wqk{kd}")
                    if kd == 0:
                        # first-needed weights: halves on both queues so the
                        # first matmul starts ~3us earlier
                        nc.sync.dma_start(
                            wq[:, 0:256], wqk_d.ap()[0:128, 0:256])
                        nc.scalar.dma_start(
                            wq[:, 256:512], wqk_d.ap()[0:128, 256:512])
                    else:
                        nc.sync.dma_start(
                            wq[:], wqk_d.ap()[kd * 128:(kd + 1) * 128, :])
                    wqk_sb.append(wq)
                    wv_t = wv_pool.tile([128, 256], bf16, tag=f"wv{kd}")
                    nc.sync.dma_start(wv_t[:], wv_d.ap()[kd * 128:(kd + 1) * 128, :])
                    wv_sb.append(wv_t)
                # xt DMAs are issued just-in-time inside the kd loop; the
                # 10-buf ring throttles in-flight transfers so the DMA
                # engines never back up (a 32-buf prefetch-ahead variant
                # measured ~15us slower: issue/descriptor backlog delayed
                # the very tiles it tried to get early).
                for c in range(NCH):
                    if c == 1:
                        for h in range(HPC):
                            nc.scalar.dma_start(
                                wo_sb[h][:], wo_d.ap()[h * 128:(h + 1) * 128, :])
                    qk_ps = [qk_ps_pool.tile([128, 512], f32, tag="qk", name="qkps")
                             for _ in range(4)]
                    v_ps = [v_ps_pool.tile([128, 256], f32, tag="v", name="vps")
                            for _ in range(4)]
                    for kd in range(NDT):
                        xt_t = xt_pool.tile([128, 512], bf16, tag="xt")
                        if c == 0 and kd == 0:
                            # two issues, two DMA engines: full tile ~2.5us
                            # sooner (sync is full of weight DMAs)
                            nc.scalar.dma_start(
                                xt_t[:, 0:256], xt_d.ap()[0:128, 0:256])
                            nc.scalar.dma_start(
                                xt_t[:, 256:512], xt_d.ap()[0:128, 256:512])
                        else:
                            eng = (nc.scalar
                                   if c < 2 or (c * NDT + kd) % 2 == 0
                                   else nc.sync)
                            eng.dma_start(
                                xt_t[:], xt_d.ap()[kd * 128:(kd + 1) * 128,
                                                   c * 512:(c + 1) * 512])
                        st, sp = kd == 0, kd == NDT - 1
                        for e in range(4):
                            nc.tensor.matmul(
                                qk_ps[e][:], wqk_sb[kd][:, e * 128:(e + 1) * 128],
                                xt_t[:], start=st, stop=sp)
                        for tl in range(4):
                            nc.tensor.matmul(
                                v_ps[tl][:],
                                xt_t[:, tl * 128:(tl + 1) * 128],
                                wv_sb[kd][:], start=st, stop=sp)
                    # alternate engines so the next chunk's first matmuls
                    # (which reuse these PSUM banks) unblock pairwise fast
                    dsts = [qt_sb[0], qt_sb[1], kt_sb[0], kt_sb[1]]
                    for e in range(4):
                        dst = dsts[e][:, c * 512:(c + 1) * 512]
                        if e % 2 == 0:
                            nc.vector.tensor_copy(dst, qk_ps[e][:])
                        else:
                            nc.scalar.copy(dst, qk_ps[e][:])
                    for tl in range(4):
                        tok = c * 4 + tl
                        dst = v_sb[:, tok * 256:(tok + 1) * 256]
                        if tl % 2 == 0:
                            nc.scalar.copy(dst, v_ps[tl][:])
                        else:
                            nc.vector.tensor_copy(dst, v_ps[tl][:])

            # ---------------- phase 2 + 3: attention + projection ----------------
            from concourse import bass_isa

            with ExitStack() as p2:
                e_pool = p2.enter_context(tc.tile_pool(name="e", bufs=8))
                acc_pool = p2.enter_context(tc.tile_pool(name="acc", bufs=4))
                rcp_pool = p2.enter_context(tc.tile_pool(name="rcp", bufs=2))
                osb_pool = p2.enter_context(tc.tile_pool(name="osb", bufs=3))
                msk_pool = p2.enter_context(tc.tile_pool(name="msk", bufs=1))
                ysb_pool = p2.enter_context(tc.tile_pool(name="ysb", bufs=4))
                s_ps_pool = p2.enter_context(
                    tc.tile_pool(name="ps_s", bufs=2, space="PSUM"))
                o_ps_pool = p2.enter_context(
                    tc.tile_pool(name="ps_o", bufs=2, space="PSUM"))
                y_ps_pool = p2.enter_context(
                    tc.tile_pool(name="ps_y", bufs=2, space="PSUM"))

                # mask tile cache keyed by block content
                mask_tiles = {}

                def mask_tile(j, qb, m0, m1):
                    key = m2[qb * QB + m0:qb * QB + m1,
                             j * 128:(j + 1) * 128].tobytes()
                    t = mask_tiles.get(key)
                    if t is None:
                        t = msk_pool.tile([128, QB], bf16, name=f"mask{len(mask_tiles)}",
                                          tag=f"m{len(mask_tiles)}")
                        nc.gpsimd.dma_start(
                            t[:, 0:m1 - m0],
                            mt_d.ap()[j * 128:(j + 1) * 128,
                                      qb * QB + m0:qb * QB + m1])
                        mask_tiles[key] = t
                    return t

                # Attention: the two head-streams of a batch are emitted
                # entry-interleaved (h0/h1 alternating per k-tile) so the PE
                # queue never blocks on one stream's exp. The softmax
                # denominator never touches the PE: exp partial sums
                # accumulate on DVE/GPSIMD chains, the cross-partition sum +
                # broadcast is one GPSIMD partition_all_reduce, 1/d is a fast
                # custom-DVE reciprocal, and the o*(1/d) multiply reads the
                # result directly. Each q-block's tail is software-pipelined
                # into the next block's matmul rounds so the PE stays fed;
                # projection tiles drain one per round behind that.
                class QbStream:
                    def __init__(self, b, h, qb):
                        self.b, self.h, self.qb = b, h, qb
                        self.tb = b * S
                        self.entries = blocks[qb]
                        self.ne = len(self.entries)
                        self.accA = acc_pool.tile([128, QB], f32r,
                                                  tag="accA", name="accA")
                        self.accB = acc_pool.tile([128, QB], f32r,
                                                  tag="accB", name="accB")
                        self.startA = None  # leftmost initialized col
                        self.startB = None
                        self.o_ps = o_ps_pool.tile([128, QB], f32, tag="o",
                                                   name="ops")
                        self.qcol = self.tb + qb * QB
                        self.pend = []  # up to 2 groups' PVs in flight
                        self.gi = 0

                    def _acc(self, gi, q0c, esl):
                        use_g = gi % 3 == 2  # GPSIMD chain (slower)
                        eng = nc.gpsimd if use_g else nc.vector
                        acc = self.accA if use_g else self.accB
                        st = self.startA if use_g else self.startB
                        if st is None:
                            eng.tensor_copy(acc[:, q0c:QB], esl)
                            st = q0c
                        elif q0c < st:
                            eng.tensor_copy(acc[:, q0c:st],
                                            esl[:, 0:st - q0c])
                            eng.tensor_add(acc[:, st:QB], acc[:, st:QB],
                                           esl[:, st - q0c:])
                            st = q0c
                        else:
                            eng.tensor_add(acc[:, q0c:QB], acc[:, q0c:QB],
                                           esl)
                        if use_g:
                            self.startA = st
                        else:
                            self.startB = st

                    def s_and_exp(self, grp, g0):
                        s_ps = s_ps_pool.tile([128, 2 * QB], f32, tag="s",
                                              name="sps")
                        for idx, (j, q0c, m0, m1) in enumerate(grp):
                            nc.tensor.matmul(
                                s_ps[:, idx * QB + q0c:(idx + 1) * QB],
                                kt_sb[self.h][:, self.tb + j * 128:
                                              self.tb + (j + 1) * 128],
                                qt_sb[self.h][:, self.qcol + q0c:
                                              self.qcol + QB],
                                start=True, stop=True)
                        e_sb = e_pool.tile([128, 2 * QB], bf16, tag="e",
                                           name="esb")
                        if len(grp) == 2 and all(e[1] == 0 for e in grp):
                            nc.scalar.activation(e_sb[:], s_ps[:], EXP,
                                                 scale=scale)
                        else:
                            for idx, (j, q0c, m0, m1) in enumerate(grp):
                                lo = idx * QB + q0c
                                hi = (idx + 1) * QB
                                nc.scalar.activation(
                                    e_sb[:, lo:hi], s_ps[:, lo:hi], EXP,
                                    scale=scale)
                        for idx, (j, q0c, m0, m1) in enumerate(grp):
                            if m0 is not None:
                                mtile = mask_tile(j, self.qb, m0, m1)
                                lo = idx * QB + m0
                                hi = idx * QB + m1
                                nc.vector.tensor_mul(
                                    e_sb[:, lo:hi], e_sb[:, lo:hi],
                                    mtile[:, 0:m1 - m0])
                        for idx, (j, q0c, m0, m1) in enumerate(grp):
                            self._acc(g0 + idx, q0c,
                                      e_sb[:, idx * QB + q0c:(idx + 1) * QB])
                        return e_sb

                    def pv(self, grp, g0, e_sb):
                        for idx, (j, q0c, m0, m1) in enumerate(grp):
                            gi = g0 + idx
                            nc.tensor.matmul(
                                self.o_ps[:, q0c:QB],
                                v_sb[:, (self.b * NKT + j) * 256 + self.h * 128:
                                     (self.b * NKT + j) * 256 + (self.h + 1) * 128],
                                e_sb[:, idx * QB + q0c:(idx + 1) * QB],
                                start=(gi == 0), stop=(gi == self.ne - 1))

                    def step(self):
                        # one group of 2 entries: S+exp+acc for group at
                        # gi, PV for the previous group. (A 2-deep PV lag
                        # measured ~66us slower: it defers each block's
                        # o_ps completion by two rounds and the tail chain
                        # compounds across blocks.)
                        if self.gi < self.ne:
                            grp = self.entries[self.gi:self.gi + 2]
                            e_sb = self.s_and_exp(grp, self.gi)
                            if len(self.pend) == 1:
                                self.pv(*self.pend.pop(0))
                            self.pend.append((grp, self.gi, e_sb))
                            self.gi += len(grp)
                            return True
                        return False

                    def flush(self):
                        # merge the two acc chains. The last PV is NOT
                        # drained here: its exp is only ~1 ACT slot old, so
                        # emitting it now would stall the PE at every block
                        # boundary — tail_a drains it a round later.
                        if self.startA is not None:
                            sa = max(self.startA, self.startB)
                            if self.startA < self.startB:
                                # accA covers a wider prefix: swap roles
                                nc.vector.tensor_add(
                                    self.accA[:, sa:QB], self.accA[:, sa:QB],
                                    self.accB[:, sa:QB])
                                self.merged = self.accA
                            else:
                                nc.vector.tensor_add(
                                    self.accB[:, sa:QB], self.accB[:, sa:QB],
                                    self.accA[:, sa:QB])
                                self.merged = self.accB
                        else:
                            self.merged = self.accB

                    def tail_a(self):
                        while self.pend:
                            self.pv(*self.pend.pop(0))
                        self.o_sb = osb_pool.tile([128, QB], bf16, tag="osb",
                                                  name="osb")
                        nc.scalar.copy(self.o_sb[:], self.o_ps[:])
                        # cross-partition sum of exp via a K=1 ones matmul
                        # (deps long satisfied when the in-order PE queue
                        # reaches it), then 18-bit 1/d on DVE.
                        self.d_ps = y_ps_pool.tile([128, QB], f32, tag="y",
                                                   name="dps")
                        nc.tensor.matmul(self.d_ps[0:1, :], ones_col[:],
                                         self.merged[:], start=True, stop=True)
                        rcp_f = rcp_pool.tile([1, QB], f32, tag="rcpf",
                                              name="rcpf")
                        nc.vector.reciprocal_approx_fast(rcp_f[:],
                                                         self.d_ps[0:1, :])
                        # matmul wants a producer that rounds to f32r
                        # (ACT Reciprocal would fuse these two ops but bass
                        # rejects it for accuracy)
                        self.rcp = rcp_pool.tile([1, QB], f32r, tag="rcp",
                                                 name="rcp")
                        nc.vector.tensor_copy(self.rcp[:], rcp_f[:])

                    def tail_b(self):
                        # broadcast 1/d across partitions with a K=1 matmul.
                        # Fresh tile, NOT d_ps: reusing d_ps would hold a
                        # y-ring slot from tail_a to here (~2 rounds), and
                        # any proj tile drained in between stalls the PE on
                        # that slot. Both tiles now free within ~1us.
                        bc_ps = y_ps_pool.tile([128, QB], f32, tag="y",
                                               name="bcps")
                        nc.tensor.matmul(bc_ps[:], ones_row[:],
                                         self.rcp[:], start=True, stop=True)
                        # the direct PSUM-operand multiply shows 1.8-2.1us
                        # slices, but that is mostly overlappable latency:
                        # splitting it into a PSUM->SBUF cast + 16-bit
                        # multiply measured ~10us SLOWER end-to-end (two
                        # dependency hops instead of one on the tail path)
                        nc.vector.tensor_mul(
                            at_sb[self.h][:, self.qcol:self.qcol + QB],
                            self.o_sb[:], bc_ps[:])

                def emit_proj_tile(b, tt, fine=False):
                    trow = (b * NKT + tt) * 128
                    for ch in range(4):
                        y_ps = y_ps_pool.tile([128, 512], f32, tag="y",
                                              name="yps")
                        for hh in range(HPC):
                            nc.tensor.matmul(
                                y_ps[:],
                                at_sb[hh][:, trow:trow + 128],
                                wo_sb[hh][:, ch * 512:(ch + 1) * 512],
                                start=(hh == 0), stop=(hh == HPC - 1))
                        y_sb = ysb_pool.tile([128, 512], bf16, tag="ysb",
                                             name="ysb")
                        if ch % 2 == 0:
                            nc.scalar.copy(y_sb[:], y_ps[:])
                        else:
                            nc.vector.tensor_copy(y_sb[:], y_ps[:])
                        if not fine:
                            # y rides the sync queue only — an issue costs
                            # ~650ns of queue time and the scalar (ACT)
                            # queue must stay free for the exp cadence.
                            nc.sync.dma_start(
                                y_d.ap()[trow:trow + 128,
                                         ch * 512:(ch + 1) * 512], y_sb[:])
                        else:
                            # closing tiles: nothing left for the ACT queue
                            # to feed, so spread half-width pieces over both
                            # issue queues to shorten the drain.
                            for hf in range(2):
                                eng = nc.sync if hf == 0 else nc.scalar
                                lo = ch * 512 + hf * 256
                                eng.dma_start(
                                    y_d.ap()[trow:trow + 128, lo:lo + 256],
                                    y_sb[:, hf * 256:hf * 256 + 256])

                proj_queue = []  # (b, tt) pending projection tiles

                def drain_proj(n, fine=False):
                    for _ in range(min(n, len(proj_queue))):
                        emit_proj_tile(*proj_queue.pop(0), fine=fine)

                import os as _os
                PIPELINE_TAILS = _os.environ.get("PIPELINE_TAILS", "1") == "1"
                pending = None  # prior q-block awaiting its pipelined tail
                for b in range(B):
                    for qb in range(NQB):
                        streams = [QbStream(b, h, qb) for h in range(HPC)]
                        alive = True
                        nstep = 0
                        while alive:
                            alive = False
                            for st in streams:
                                if st.step():
                                    alive = True
                            nstep += 1
                            # drain BEFORE the tails: the proj tile's deps
                            # are rounds old, the tail matmuls' (last exp,
                            # fresh rcp) are not — ready work must sit
                            # ahead of dependent work in the in-order PE
                            # queue or the PE stalls with work available.
                            # Boundary rounds get double cover when the
                            # queue has a reserve.
                            if nstep <= 2 and len(proj_queue) >= 3:
                                drain_proj(2)
                            else:
                                drain_proj(1)
                            if pending is not None:
                                if nstep == 1:
                                    for ps in pending[0]:
                                        ps.tail_a()
                                elif nstep == 2:
                                    for ps in pending[0]:
                                        ps.tail_b()
                                    pb, pqb = pending[1], pending[2]
                                    proj_queue.extend(
                                        (pb, pqb * 4 + t4) for t4 in range(4))
                                    pending = None
                        for st in streams:
                            st.flush()
                        if PIPELINE_TAILS:
                            pending = (streams, b, qb)
                        else:
                            for ps in streams:
                                ps.tail_a()
                            for ps in streams:
                                ps.tail_b()
                            proj_queue.extend(
                                (b, qb * 4 + t4) for t4 in range(4))
                if pending is not None:
                    # last q-block's tail: keep the PE on proj work meanwhile
                    drain_proj(1)
                    for ps in pending[0]:
                        ps.tail_a()
                    drain_proj(1)
                    for ps in pending[0]:
                        ps.tail_b()
                    proj_queue.extend(
                        (pending[1], pending[2] * 4 + t4) for t4 in range(4))
                while len(proj_queue) > 2:
                    drain_proj(1)
                drain_proj(2, fine=True)

    nc.compile()
    return nc


last_results = None  # set when KERNEL_TRACE=1 (profiling from test harness)


def kernel(x, mask, W_qkv, W_o):
    import os
    import ml_dtypes
    from concourse.bass_utils import run_bass_kernel_spmd

    bf = ml_dtypes.bfloat16
    x = np.asarray(x, dtype=np.float32)
    mask_np = np.asarray(mask).astype(bool)
    W_qkv = np.asarray(W_qkv, dtype=np.float32)
    W_o = np.asarray(W_o, dtype=np.float32)
    m2 = np.broadcast_to(mask_np, (1, 1, S, S))[0, 0]

    key = m2.tobytes()
    nc = _cache.get(key)
    if nc is None:
        nc = _build(m2)
        _cache[key] = nc

    xt = np.ascontiguousarray(x.reshape(T, D).T.astype(bf))  # [D, T]
    mt = np.ascontiguousarray(m2.T.astype(bf))               # [k, q]

    in_maps = []
    for c in range(NCORES):
        hA, hB = HPC * c, HPC * c + 1
        q_rows = list(range(hA * DK, (hA + 1) * DK)) + \
                 list(range(hB * DK, (hB + 1) * DK))
        k_rows = [D + r for r in q_rows]
        v_rows = [2 * D + r for r in q_rows]
        wqk = np.ascontiguousarray(W_qkv[q_rows + k_rows, :].T.astype(bf))
        wv = np.ascontiguousarray(W_qkv[v_rows, :].T.astype(bf))   # [D, 256]
        wo = np.ascontiguousarray(W_o[:, q_rows].T.astype(bf))     # [256, D]
        in_maps.append({"xt": xt, "wqk": wqk, "wv": wv, "wo": wo, "mt": mt})

    trace = bool(os.environ.get("KERNEL_TRACE"))
    res = run_bass_kernel_spmd(nc, in_maps, core_ids=list(range(NCORES)),
                               trace=trace)
    if trace:
        global last_results
        last_results = res
    y = res.results[0]["y"].astype(np.float32)
    for c in range(1, NCORES):
        y += res.results[c]["y"].astype(np.float32)
    return y.reshape(B, S, D)

